# revision 32
# baseline (speedup 1.0000x reference)
"""nn_GatedRecurrentBlock — hand-written Bass/Tile kernel for 8x Trainium2 cores.

Strategy (data-parallel over batch, 1024 rows/core, 2 row-blocks of 512):
  - Host folds the attention v-proj + out-proj into one matrix (softmax over a
    single key == 1, so attn == v), folds g1/g2 into the weight matrices, pads
    HIDDEN 5324 -> 5376, and pre-packs every weight into consumption-ordered
    [Mtiles, 128, Ktiles, 128] bf16 lhsT tiles (contiguous DMA).
  - Activations live feature-major ([feat, row]) on chip so no on-device
    transposes are needed; host ships x/state pre-transposed bf16 and
    re-transposes the fp32 output.
  - RMSNorm: squares on ScalarE, cross-partition reduce via ones-matmul on
    TensorE, per-row 1/norm broadcast via gpsimd partition_broadcast, scale
    folded into PSUM eviction (attn) or a single tensor_tensor (ffn).
"""

import numpy as np
import ml_dtypes
import bass_rust

import concourse.bass as bass
import concourse.mybir as mybir
import concourse.tile as tile
from concourse.bass_utils import run_bass_kernel_spmd

P = 128
DIM = 2048
KC = DIM // P              # 16 feature chunks
HID = 5324
HIDP = 5376                # padded hidden
MH = HIDP // P             # 42 hidden chunks
BATCH = 8192
NCORES = 8
RPC = BATCH // NCORES      # 1024 rows per core
R = 512                    # rows per block (one PSUM bank of fp32)
NB = RPC // R              # 2 blocks
EPS = 1e-6
ISQ = DIM ** -0.5
SQH = 0.7071

BF16 = mybir.dt.bfloat16
F32 = mybir.dt.float32
nbf = ml_dtypes.bfloat16

AF = mybir.ActivationFunctionType
ALU = mybir.AluOpType


def _pack_lhsT(w):
    """[K, M] fp32 -> [M//P, P, K//P, P] bf16, tile-contiguous in consumption order."""
    K, M = w.shape
    return np.ascontiguousarray(
        w.astype(nbf).reshape(K // P, P, M // P, P).transpose(2, 1, 0, 3))


def _opname(inst):
    try:
        return inst.opcode.name
    except AttributeError:
        return str(inst.opcode)


def _split_waits(nc, caps={}, cap_default=1):
    """This neuronxcc build caps sync-wait commands at 1 per instruction.
    Tile's wait assignment can exceed that; spill excess waits onto preceding
    same-engine NOPs (sequencers execute waits in program order, so this is
    semantics-preserving)."""
    for f in nc.m.functions:
        for bb in f.blocks:
            insts = list(bb.instructions)
            out = []
            changed = False
            for inst in insts:
                si = inst.sync_info
                waits = list(si.on_wait) if si is not None and si.on_wait else []
                cap = caps.get(_opname(inst), cap_default)
                if len(waits) > cap:
                    changed = True
                    keep = waits[len(waits) - cap:] if cap > 0 else []
                    spill = waits[: len(waits) - cap] if cap > 0 else waits
                    for w in spill:
                        nop = mybir.InstNoOp(name=f"I-{nc.next_id()}", ins=[], outs=[])
                        nop.engine = inst.engine
                        nop.sync_info = bass_rust.SyncInfo(on_wait=[w], on_update=[])
                        nc.register_instruction(nop, overwrite=True)
                        out.append(nop)
                    si.on_wait = keep
                    inst.sync_info = si
                out.append(inst)
            if changed:
                bb.instructions = out
    return nc


def build_program(use_bvo: bool, use_gb: bool) -> bass.Bass:
    nc = bass.Bass("TRN2", debug=False)

    xT = nc.dram_tensor("xT", [KC, P, RPC], BF16, kind="ExternalInput")
    sT = nc.dram_tensor("sT", [KC, P, RPC], BF16, kind="ExternalInput")
    wvo = nc.dram_tensor("wvo", [KC, P, KC, P], BF16, kind="ExternalInput")
    w12 = nc.dram_tensor("w12", [MH, P, 2, KC, P], BF16, kind="ExternalInput")
    w3 = nc.dram_tensor("w3", [KC, P, MH, P], BF16, kind="ExternalInput")
    wg = nc.dram_tensor("wg", [KC, P, 2 * KC, P], BF16, kind="ExternalInput")
    if use_bvo:
        bvo = nc.dram_tensor("bvo", [P, KC], F32, kind="ExternalInput")
    if use_gb:
        gb = nc.dram_tensor("gb", [P, KC], F32, kind="ExternalInput")
    yT = nc.dram_tensor("yT", [KC, P, RPC], F32, kind="ExternalOutput")

    with tile.TileContext(nc) as tc:
        with (
            tc.tile_pool(name="const", bufs=1) as constp,
            tc.tile_pool(name="pA", bufs=4) as pA,
            tc.tile_pool(name="pB", bufs=1) as pB,
            tc.tile_pool(name="pC", bufs=1) as pC,
            tc.tile_pool(name="pS", bufs=2) as pS,
            tc.tile_pool(name="pG", bufs=1) as pG,
            tc.tile_pool(name="pW", bufs=2) as pW,
            tc.tile_pool(name="small", bufs=2) as psm,
            tc.tile_pool(name="mm", bufs=6, space="PSUM") as pmm,
            tc.tile_pool(name="ssp", bufs=2, space="PSUM") as pss,
        ):
            ones = constp.tile([P, 1], BF16)
            nc.vector.memset(ones, 1.0)
            # the DIM**-0.5 rmsnorm scale rides on the broadcast outer-product
            ones_row = constp.tile([1, P], F32)
            nc.vector.memset(ones_row, ISQ)
            if use_bvo:
                bvo_sb = constp.tile([P, KC], F32)
                nc.sync.dma_start(out=bvo_sb, in_=bvo.ap())
            if use_gb:
                gb_sb = constp.tile([P, KC], F32)
                nc.sync.dma_start(out=gb_sb, in_=gb.ap())

            def emit_input_dma(b):
                """DMAs for block b's inputs, split so single transfers don't
                hog a DMA lane ahead of same-phase weight prefetches. Order
                xt/st half-by-half so res halves can start early."""
                rows = slice(b * R, (b + 1) * R)
                xt = pC.tile([P, KC, R], BF16, tag="C")
                st = pS.tile([P, KC, R], BF16, tag="S")
                parts = []
                for h in range(4):
                    for dst, src in ((xt, xT), (st, sT)):
                        cs = slice(h * (KC // 4), (h + 1) * (KC // 4))
                        parts.append(lambda dst=dst, src=src, cs=cs, rows=rows:
                                     nc.scalar.dma_start(
                                         out=dst[:, cs],
                                         in_=src.ap()[cs, :, rows].rearrange("c p r -> p c r")))
                return xt, st, parts

            def emit_h_and_squares(b, xt, st):
                """res = (x+state)*sqrt(1/2) computed per input half; per-chunk
                squares for rmsnorm 1. Vector/Scalar only — safe to trace
                inside the previous block's gate loop."""
                res = pG.tile([P, KC, R], BF16, tag="G")
                hsq1 = [pA.tile([P, 4, R], BF16, tag="A", name=f"hsq1_{i}") for i in range(4)]
                half = KC // 2
                for h in range(2):
                    cs = slice(h * half, (h + 1) * half)
                    nc.vector.tensor_add(out=res[:, cs], in0=xt[:, cs], in1=st[:, cs])
                    nc.vector.tensor_scalar_mul(res[:, cs], res[:, cs], SQH)
                    for c in range(h * half, (h + 1) * half):
                        dst = hsq1[c // 4][:, c % 4]
                        if c % 2 == 0:
                            nc.scalar.activation(out=dst, in_=res[:, c], func=AF.Square)
                        else:
                            nc.vector.tensor_tensor(dst, res[:, c], res[:, c], ALU.mult)
                return res, hsq1

            def emit_rn(ss):
                """ss [1,R] psum -> rn [1,R] = 1/max(||.||,eps). No PE work."""
                rn = psm.tile([1, R], F32, tag="rn", bufs=1)
                nc.scalar.activation(out=rn, in_=ss, func=AF.Sqrt)
                nc.vector.tensor_scalar_max(rn, rn, EPS)
                nc.vector.reciprocal(rn, rn)
                return rn

            def emit_bcast(rn):
                """rn -> bc [P,R] f32 via ones outer-product (ISQ folded into
                ones_row). Trace this AFTER some main matmuls: the PE queue is
                in-order and this matmul waits on the rn scalar chain."""
                psb = pmm.tile([P, R], F32, tag="ps")
                nc.tensor.matmul(psb, ones_row, rn, start=True, stop=True)
                bc = psm.tile([P, R], F32, tag="bc", bufs=2)
                nc.scalar.copy(bc, psb)
                return bc

            def emit_attn(b, res, hsq1):
                # fused v+out proj; h2 = res + rn1*psum + bvo. The norm-1
                # reduce chain is traced after a 4-chunk matmul prefix (its
                # evictions deferred) so the PE never idles waiting for bc1;
                # norm-2 squares interleave and their reduce matmuls trail by
                # 2 chunks.
                h2 = pC.tile([P, KC, R], BF16, tag="C")
                ss2_box = {}
                hsq2 = [None] * KC
                bc1_box = {}
                ss2_emitted = 0

                def evict(m, ps):
                    tb = psm.tile([P, R], BF16, tag="tb", bufs=1)
                    nc.vector.tensor_tensor(tb, ps, bc1_box["bc"], ALU.mult)
                    if use_bvo:
                        nc.vector.tensor_scalar_add(tb, tb, bvo_sb[:, m:m + 1])
                    nc.vector.tensor_add(out=h2[:, m], in0=tb, in1=res[:, m])
                    hsq = psm.tile([P, R], BF16, tag="hsq", bufs=4)
                    nc.scalar.activation(out=hsq, in_=h2[:, m], func=AF.Square)
                    hsq2[m] = hsq

                def emit_ss2_upto(j):
                    nonlocal ss2_emitted
                    while ss2_emitted < j:
                        if "ss" not in ss2_box:
                            ss2_box["ss"] = pss.tile([1, R], F32, tag="ss", name="ss2")
                        mm = ss2_emitted
                        nc.tensor.matmul(ss2_box["ss"], ones, hsq2[mm],
                                         start=(mm == 0), stop=(mm == KC - 1))
                        ss2_emitted += 1

                pending = []
                rn1_box = {}
                for m in range(KC):
                    wt = pW.tile([P, KC, P], BF16, tag="wvo")
                    nc.sync.dma_start(out=wt, in_=wvo.ap()[m])
                    ps = pmm.tile([P, R], F32, tag="ps")
                    for k in range(KC):
                        nc.tensor.matmul(ps, wt[:, k], res[:, k],
                                         start=(k == 0), stop=(k == KC - 1))
                    if "bc" not in bc1_box:
                        pending.append((m, ps))
                    else:
                        evict(m, ps)
                        emit_ss2_upto(m - 1)
                    if m == 3:
                        ss1 = pss.tile([1, R], F32, tag="ss")
                        for c in range(KC):
                            nc.tensor.matmul(ss1, ones, hsq1[c // 4][:, c % 4],
                                             start=(c == 0), stop=(c == KC - 1))
                        rn1_box["rn"] = emit_rn(ss1)
                    if m == 4:
                        bc1_box["bc"] = emit_bcast(rn1_box["rn"])
                        for mm, pps in pending:
                            evict(mm, pps)
                emit_ss2_upto(KC - 2)
                return h2, ss2_box, lambda: emit_ss2_upto(KC)

            def emit_ffn_up(h2, ss2_box, finish_ss2):
                # unnormalized h2 feeds the matmuls; the rmsnorm scale is
                # applied to the PSUM outputs during eviction (keeps the
                # norm-2 chain off the PE critical path). The ss2 tail, rn
                # chain, and broadcast are woven between the first chunks'
                # matmuls so the in-order PE queue never waits on them.
                g = pG.tile([P, MH, R], BF16, tag="G")
                bc2_box = {}
                rn2_box = {}
                pending = []

                def evict(m, psa, psb):
                    bc2 = bc2_box["bc"]
                    ta = psm.tile([P, R], BF16, tag="ta")
                    nc.vector.tensor_tensor(ta, psa, bc2, ALU.mult)
                    sg = psm.tile([P, R], BF16, tag="sg")
                    nc.scalar.activation(out=sg, in_=ta, func=AF.Sigmoid)
                    nc.vector.tensor_tensor(sg, sg, ta, ALU.mult)
                    tb2 = psm.tile([P, R], BF16, tag="tb2")
                    nc.vector.tensor_tensor(tb2, psb, bc2, ALU.mult)
                    nc.vector.tensor_tensor(g[:, m], sg, tb2, ALU.mult)

                for m in range(MH):
                    wt12 = pW.tile([P, 2, KC, P], BF16, tag="w12")
                    nc.sync.dma_start(out=wt12, in_=w12.ap()[m])
                    psa = pmm.tile([P, R], F32, tag="ps")
                    for k in range(KC):
                        nc.tensor.matmul(psa, wt12[:, 0, k], h2[:, k],
                                         start=(k == 0), stop=(k == KC - 1))
                    psb = pmm.tile([P, R], F32, tag="ps")
                    for k in range(KC):
                        nc.tensor.matmul(psb, wt12[:, 1, k], h2[:, k],
                                         start=(k == 0), stop=(k == KC - 1))
                    if "bc" not in bc2_box:
                        pending.append((m, psa, psb))
                    else:
                        evict(m, psa, psb)
                    if m == 0:
                        finish_ss2()
                        rn2_box["rn"] = emit_rn(ss2_box["ss"])
                    if m == 1:
                        bc2_box["bc"] = emit_bcast(rn2_box["rn"])
                        for args in pending:
                            evict(*args)
                return g

            def emit_ffn_down(g, h2):
                cand = pB.tile([P, KC, R], BF16, tag="B")
                for m in range(KC):
                    wt3 = pW.tile([P, MH, P], BF16, tag="w3")
                    nc.sync.dma_start(out=wt3, in_=w3.ap()[m])
                    ps = pmm.tile([P, R], F32, tag="ps")
                    for k in range(MH):
                        nc.tensor.matmul(ps, wt3[:, k], g[:, k],
                                         start=(k == 0), stop=(k == MH - 1))
                    nc.vector.tensor_add(out=cand[:, m], in0=ps, in1=h2[:, m])
                return cand

            def emit_gate(b, cand, st, interleave):
                rows = slice(b * R, (b + 1) * R)
                for m in range(KC):
                    wtg = pW.tile([P, 2 * KC, P], BF16, tag="wg")
                    nc.sync.dma_start(out=wtg, in_=wg.ap()[m])
                    ps = pmm.tile([P, R], F32, tag="ps")
                    # state half first: st is ready long before cand, so the
                    # PE can start each gate chunk before FFN-down finishes.
                    for k in range(KC):
                        nc.tensor.matmul(ps, wtg[:, KC + k], st[:, k],
                                         start=(k == 0), stop=False)
                    for k in range(KC):
                        nc.tensor.matmul(ps, wtg[:, k], cand[:, k],
                                         start=False, stop=(k == KC - 1))
                    z = psm.tile([P, R], BF16, tag="z", bufs=1)
                    nc.scalar.activation(out=z, in_=ps, func=AF.Sigmoid,
                                         bias=(gb_sb[:, m:m + 1] if use_gb else 0.0))
                    d = psm.tile([P, R], BF16, tag="d", bufs=1)
                    nc.vector.tensor_tensor(d, cand[:, m], st[:, m], ALU.subtract)
                    nc.vector.tensor_tensor(d, z, d, ALU.mult)
                    y = psm.tile([P, R], F32, tag="y")
                    nc.vector.tensor_add(out=y, in0=d, in1=st[:, m])
                    nc.scalar.dma_start(out=yT.ap()[m][:, rows], in_=y)
                    if m in interleave:
                        interleave[m]()

            # ---- software-pipelined block schedule: block b+1's input DMA,
            # h, and norm squares are traced inside block b's gate loop so
            # every engine's in-order stream interleaves the two blocks.
            nxt = {}

            def start_block(b):
                xt, st, parts = emit_input_dma(b)
                for p in parts:
                    p()
                res, hsq1 = emit_h_and_squares(b, xt, st)
                return {"st": st, "res": res, "hsq1": hsq1}

            cur = start_block(0)
            for b in range(NB):
                h2, ss2_box, finish_ss2 = emit_attn(b, cur["res"], cur["hsq1"])
                g = emit_ffn_up(h2, ss2_box, finish_ss2)
                cand = emit_ffn_down(g, h2)
                interleave = {}
                if b + 1 < NB:
                    xt_n, st_n, parts = emit_input_dma(b + 1)
                    state = {}

                    def hs(bn=b + 1, xt_n=xt_n, st_n=st_n, state=state):
                        state["res"], state["hsq1"] = emit_h_and_squares(bn, xt_n, st_n)

                    for i, pfn in enumerate(parts):
                        interleave[1 + i] = pfn
                    interleave[10] = hs
                    nxt = {"st": st_n, "state": state}
                emit_gate(b, cand, cur["st"], interleave)
                if b + 1 < NB:
                    cur = {"st": nxt["st"],
                           "res": nxt["state"]["res"],
                           "hsq1": nxt["state"]["hsq1"]}

    return _split_waits(nc)


_prog_cache: dict = {}


def _get_prog(use_bvo: bool, use_gb: bool) -> bass.Bass:
    key = (use_bvo, use_gb)
    if key not in _prog_cache:
        _prog_cache[key] = build_program(use_bvo, use_gb)
    return _prog_cache[key]


def prepare_inputs(x, state, g1, g2, in_proj_w, in_proj_b, out_proj_w, out_proj_b,
                   w1, w2, w3, gate_w, gate_b):
    """Host-side folding/packing. Returns (in_maps, use_bvo, use_gb)."""
    f32 = np.float32
    x = np.asarray(x, f32); state = np.asarray(state, f32)
    g1 = np.asarray(g1, f32); g2 = np.asarray(g2, f32)
    in_proj_w = np.asarray(in_proj_w, f32); in_proj_b = np.asarray(in_proj_b, f32)
    out_proj_w = np.asarray(out_proj_w, f32); out_proj_b = np.asarray(out_proj_b, f32)
    w1 = np.asarray(w1, f32); w2 = np.asarray(w2, f32); w3 = np.asarray(w3, f32)
    gate_w = np.asarray(gate_w, f32); gate_b = np.asarray(gate_b, f32)

    wv = in_proj_w[2 * DIM:]
    bv = in_proj_b[2 * DIM:]
    # attn == v (softmax over one key); fold v-proj + out-proj (and g1) together
    W_vo = (wv * g1[None, :]).T @ out_proj_w.T          # [K=2048, M=2048]
    b_vo = bv @ out_proj_w.T + out_proj_b               # [2048]
    W1 = np.zeros((DIM, HIDP), f32); W1[:, :HID] = (w1 * g2[None, :]).T
    W2 = np.zeros((DIM, HIDP), f32); W2[:, :HID] = (w2 * g2[None, :]).T
    W3 = np.zeros((HIDP, DIM), f32); W3[:HID] = w3.T
    WG = np.concatenate([gate_w[:, :DIM].T, gate_w[:, DIM:].T], axis=0)  # [4096, 2048]

    weights = {
        "wvo": _pack_lhsT(W_vo),
        "w12": np.ascontiguousarray(
            np.stack([_pack_lhsT(W1), _pack_lhsT(W2)], axis=2)),
        "w3": _pack_lhsT(W3),
        "wg": _pack_lhsT(WG),
    }
    use_bvo = bool(np.any(b_vo))
    use_gb = bool(np.any(gate_b))
    if use_bvo:
        weights["bvo"] = np.ascontiguousarray(b_vo.reshape(KC, P).T)
    if use_gb:
        weights["gb"] = np.ascontiguousarray(gate_b.reshape(KC, P).T)

    in_maps = []
    for c in range(NCORES):
        rs = slice(c * RPC, (c + 1) * RPC)
        m = dict(weights)
        m["xT"] = np.ascontiguousarray(x[rs].astype(nbf).T).reshape(KC, P, RPC)
        m["sT"] = np.ascontiguousarray(state[rs].astype(nbf).T).reshape(KC, P, RPC)
        in_maps.append(m)
    return in_maps, use_bvo, use_gb


def run(inputs: dict, trace: bool = False, trace_cores=None):
    in_maps, use_bvo, use_gb = prepare_inputs(**inputs)
    nc = _get_prog(use_bvo, use_gb)
    res = run_bass_kernel_spmd(
        nc, in_maps, core_ids=list(range(NCORES)),
        trace=trace, trace_cores=trace_cores)
    out = np.empty((BATCH, DIM), np.float32)
    for c in range(NCORES):
        yt = res.results[c]["yT"].reshape(DIM, RPC)
        out[c * RPC:(c + 1) * RPC] = yt.T
    return out, res


def kernel(**inputs) -> np.ndarray:
    out, _ = run(inputs, trace=False)
    return out


# revision 33
# speedup vs baseline: 1.0278x; 1.0278x over previous
"""nn_GatedRecurrentBlock — hand-written Bass/Tile kernel for 8x Trainium2 cores.

Strategy (data-parallel over batch, 1024 rows/core, 2 row-blocks of 512):
  - Host folds the attention v-proj + out-proj into one matrix (softmax over a
    single key == 1, so attn == v), folds g1/g2 into the weight matrices, pads
    HIDDEN 5324 -> 5376, and pre-packs every weight into consumption-ordered
    [Mtiles, 128, Ktiles, 128] bf16 lhsT tiles (contiguous DMA).
  - Activations live feature-major ([feat, row]) on chip so no on-device
    transposes are needed; host ships x/state pre-transposed bf16 and
    re-transposes the fp32 output.
  - RMSNorm: squares on Scalar/Vector engines, cross-partition reduce and
    per-row 1/norm broadcast via ones-matmuls on TensorE, scale applied to
    PSUM outputs during eviction (norm chains stay off the PE critical path).
  - Two row-blocks are software-pipelined at trace level (engine streams are
    in-order): block b+1's input DMA + h + squares are traced inside block
    b's gate loop.
"""

import numpy as np
import ml_dtypes
import bass_rust

import concourse.bass as bass
import concourse.mybir as mybir
import concourse.tile as tile
from concourse.bass_utils import run_bass_kernel_spmd

P = 128
DIM = 2048
KC = DIM // P              # 16 feature chunks
HID = 5324
HIDP = 5376                # padded hidden
MH = HIDP // P             # 42 hidden chunks
BATCH = 8192
NCORES = 8
RPC = BATCH // NCORES      # 1024 rows per core
R = 512                    # rows per block (one PSUM bank of fp32)
NB = RPC // R              # 2 blocks
EPS = 1e-6
ISQ = DIM ** -0.5
SQH = 0.7071

BF16 = mybir.dt.bfloat16
F32 = mybir.dt.float32
nbf = ml_dtypes.bfloat16

AF = mybir.ActivationFunctionType
ALU = mybir.AluOpType


def _pack_lhsT(w):
    """[K, M] fp32 -> [M//P, P, K//P, P] bf16, tile-contiguous in consumption order."""
    K, M = w.shape
    return np.ascontiguousarray(
        w.astype(nbf).reshape(K // P, P, M // P, P).transpose(2, 1, 0, 3))


def _opname(inst):
    try:
        return inst.opcode.name
    except AttributeError:
        return str(inst.opcode)


def _split_waits(nc, caps={}, cap_default=1):
    """This neuronxcc build caps sync-wait commands at 1 per instruction.
    Tile's wait assignment can exceed that; spill excess waits onto preceding
    same-engine NOPs (sequencers execute waits in program order, so this is
    semantics-preserving)."""
    for f in nc.m.functions:
        for bb in f.blocks:
            insts = list(bb.instructions)
            out = []
            changed = False
            for inst in insts:
                si = inst.sync_info
                waits = list(si.on_wait) if si is not None and si.on_wait else []
                cap = caps.get(_opname(inst), cap_default)
                if len(waits) > cap:
                    changed = True
                    keep = waits[len(waits) - cap:] if cap > 0 else []
                    spill = waits[: len(waits) - cap] if cap > 0 else waits
                    for w in spill:
                        nop = mybir.InstNoOp(name=f"I-{nc.next_id()}", ins=[], outs=[])
                        nop.engine = inst.engine
                        nop.sync_info = bass_rust.SyncInfo(on_wait=[w], on_update=[])
                        nc.register_instruction(nop, overwrite=True)
                        out.append(nop)
                    si.on_wait = keep
                    inst.sync_info = si
                out.append(inst)
            if changed:
                bb.instructions = out
    return nc


def build_program(use_bvo: bool, use_gb: bool) -> bass.Bass:
    nc = bass.Bass("TRN2", debug=False)

    xT = nc.dram_tensor("xT", [KC, P, RPC], BF16, kind="ExternalInput")
    sT = nc.dram_tensor("sT", [KC, P, RPC], BF16, kind="ExternalInput")
    wvo = nc.dram_tensor("wvo", [KC, P, KC, P], BF16, kind="ExternalInput")
    w12 = nc.dram_tensor("w12", [MH, P, 2, KC, P], BF16, kind="ExternalInput")
    w3 = nc.dram_tensor("w3", [KC, P, MH, P], BF16, kind="ExternalInput")
    wg = nc.dram_tensor("wg", [KC, P, 2 * KC, P], BF16, kind="ExternalInput")
    if use_bvo:
        bvo = nc.dram_tensor("bvo", [P, KC], F32, kind="ExternalInput")
    if use_gb:
        gb = nc.dram_tensor("gb", [P, KC], F32, kind="ExternalInput")
    yT = nc.dram_tensor("yT", [KC, P, RPC], F32, kind="ExternalOutput")

    with tile.TileContext(nc) as tc:
        with (
            tc.tile_pool(name="const", bufs=1) as constp,
            tc.tile_pool(name="pA", bufs=4) as pA,
            tc.tile_pool(name="pB", bufs=1) as pB,
            tc.tile_pool(name="pC", bufs=1) as pC,
            tc.tile_pool(name="pS", bufs=2) as pS,
            tc.tile_pool(name="pG", bufs=1) as pG,
            tc.tile_pool(name="pW", bufs=2) as pW,
            tc.tile_pool(name="small", bufs=2) as psm,
            tc.tile_pool(name="mm", bufs=6, space="PSUM") as pmm,
            tc.tile_pool(name="ssp", bufs=2, space="PSUM") as pss,
        ):
            ones = constp.tile([P, 1], BF16)
            nc.vector.memset(ones, 1.0)
            # the DIM**-0.5 rmsnorm scale rides on the broadcast outer-product
            ones_row = constp.tile([1, P], F32)
            nc.vector.memset(ones_row, ISQ)
            if use_bvo:
                bvo_sb = constp.tile([P, KC], F32)
                nc.sync.dma_start(out=bvo_sb, in_=bvo.ap())
            if use_gb:
                gb_sb = constp.tile([P, KC], F32)
                nc.sync.dma_start(out=gb_sb, in_=gb.ap())

            def emit_input_dma(b):
                """DMAs for block b's inputs, split so single transfers don't
                hog a DMA lane ahead of same-phase weight prefetches. Order
                xt/st half-by-half so res halves can start early."""
                rows = slice(b * R, (b + 1) * R)
                xt = pC.tile([P, KC, R], BF16, tag="C")
                st = pS.tile([P, KC, R], BF16, tag="S")
                parts = []
                for h in range(2):
                    for dst, src in ((xt, xT), (st, sT)):
                        cs = slice(h * (KC // 2), (h + 1) * (KC // 2))
                        parts.append(lambda dst=dst, src=src, cs=cs, rows=rows:
                                     nc.sync.dma_start(
                                         out=dst[:, cs],
                                         in_=src.ap()[cs, :, rows].rearrange("c p r -> p c r")))
                return xt, st, parts

            def emit_h_and_squares(b, xt, st):
                """res = (x+state)*sqrt(1/2) computed per input half; per-chunk
                squares for rmsnorm 1. Vector/Scalar only — safe to trace
                inside the previous block's gate loop."""
                res = pG.tile([P, KC, R], BF16, tag="G")
                hsq1 = [pA.tile([P, 4, R], BF16, tag="A", name=f"hsq1_{i}") for i in range(4)]
                half = KC // 2
                for h in range(2):
                    cs = slice(h * half, (h + 1) * half)
                    nc.vector.tensor_add(out=res[:, cs], in0=xt[:, cs], in1=st[:, cs])
                    nc.vector.tensor_scalar_mul(res[:, cs], res[:, cs], SQH)
                    for c in range(h * half, (h + 1) * half):
                        dst = hsq1[c // 4][:, c % 4]
                        if c % 2 == 0:
                            nc.scalar.activation(out=dst, in_=res[:, c], func=AF.Square)
                        else:
                            nc.vector.tensor_tensor(dst, res[:, c], res[:, c], ALU.mult)
                return res, hsq1

            def emit_rn(ss):
                """ss [1,R] psum -> rn [1,R] = 1/max(||.||,eps). No PE work."""
                rn = psm.tile([1, R], F32, tag="rn", bufs=1)
                nc.scalar.activation(out=rn, in_=ss, func=AF.Sqrt)
                nc.vector.tensor_scalar_max(rn, rn, EPS)
                nc.vector.reciprocal(rn, rn)
                return rn

            def emit_bcast(rn):
                """rn -> bc [P,R] f32 via ones outer-product (ISQ folded into
                ones_row). Trace this AFTER some main matmuls: the PE queue is
                in-order and this matmul waits on the rn scalar chain."""
                psb = pmm.tile([P, R], F32, tag="ps")
                nc.tensor.matmul(psb, ones_row, rn, start=True, stop=True)
                bc = psm.tile([P, R], F32, tag="bc", bufs=2)
                nc.scalar.copy(bc, psb)
                return bc

            def emit_attn(b, res, hsq1):
                # fused v+out proj; h2 = res + rn1*psum + bvo. The norm-1
                # reduce chain is traced after a 4-chunk matmul prefix (its
                # evictions deferred) so the PE never idles waiting for bc1;
                # norm-2 squares interleave and their reduce matmuls trail by
                # 2 chunks.
                h2 = pC.tile([P, KC, R], BF16, tag="C")
                ss2_box = {}
                hsq2 = [None] * KC
                bc1_box = {}
                ss2_emitted = 0

                def evict(m, ps):
                    tb = psm.tile([P, R], BF16, tag="tb", bufs=1)
                    nc.vector.tensor_tensor(tb, ps, bc1_box["bc"], ALU.mult)
                    if use_bvo:
                        nc.vector.tensor_scalar_add(tb, tb, bvo_sb[:, m:m + 1])
                    nc.vector.tensor_add(out=h2[:, m], in0=tb, in1=res[:, m])
                    hsq = psm.tile([P, R], BF16, tag="hsq", bufs=4)
                    nc.scalar.activation(out=hsq, in_=h2[:, m], func=AF.Square)
                    hsq2[m] = hsq

                def emit_ss2_upto(j):
                    nonlocal ss2_emitted
                    while ss2_emitted < j:
                        if "ss" not in ss2_box:
                            ss2_box["ss"] = pss.tile([1, R], F32, tag="ss", name="ss2")
                        mm = ss2_emitted
                        nc.tensor.matmul(ss2_box["ss"], ones, hsq2[mm],
                                         start=(mm == 0), stop=(mm == KC - 1))
                        ss2_emitted += 1

                pending = []
                rn1_box = {}
                for m in range(KC):
                    wt = pW.tile([P, KC, P], BF16, tag="wvo")
                    nc.sync.dma_start(out=wt, in_=wvo.ap()[m])
                    ps = pmm.tile([P, R], F32, tag="ps")
                    for k in range(KC):
                        nc.tensor.matmul(ps, wt[:, k], res[:, k],
                                         start=(k == 0), stop=(k == KC - 1))
                    if "bc" not in bc1_box:
                        pending.append((m, ps))
                    else:
                        evict(m, ps)
                        emit_ss2_upto(m - 1)
                    if m == 3:
                        ss1 = pss.tile([1, R], F32, tag="ss")
                        for c in range(KC):
                            nc.tensor.matmul(ss1, ones, hsq1[c // 4][:, c % 4],
                                             start=(c == 0), stop=(c == KC - 1))
                        rn1_box["rn"] = emit_rn(ss1)
                    if m == 4:
                        bc1_box["bc"] = emit_bcast(rn1_box["rn"])
                        for mm, pps in pending:
                            evict(mm, pps)
                emit_ss2_upto(KC - 2)
                return h2, ss2_box, lambda: emit_ss2_upto(KC)

            def emit_ffn_up(h2, ss2_box, finish_ss2):
                # unnormalized h2 feeds the matmuls; the rmsnorm scale is
                # applied to the PSUM outputs during eviction (keeps the
                # norm-2 chain off the PE critical path). The ss2 tail, rn
                # chain, and broadcast are woven between the first chunks'
                # matmuls so the in-order PE queue never waits on them.
                g = pG.tile([P, MH, R], BF16, tag="G")
                bc2_box = {}
                rn2_box = {}
                pending = []

                def evict(m, psa, psb):
                    bc2 = bc2_box["bc"]
                    ta = psm.tile([P, R], BF16, tag="ta")
                    nc.vector.tensor_tensor(ta, psa, bc2, ALU.mult)
                    sg = psm.tile([P, R], BF16, tag="sg")
                    nc.scalar.activation(out=sg, in_=ta, func=AF.Sigmoid)
                    nc.vector.tensor_tensor(sg, sg, ta, ALU.mult)
                    tb2 = psm.tile([P, R], BF16, tag="tb2")
                    nc.vector.tensor_tensor(tb2, psb, bc2, ALU.mult)
                    nc.vector.tensor_tensor(g[:, m], sg, tb2, ALU.mult)

                for m in range(MH):
                    wt12 = pW.tile([P, 2, KC, P], BF16, tag="w12")
                    nc.sync.dma_start(out=wt12, in_=w12.ap()[m])
                    psa = pmm.tile([P, R], F32, tag="ps")
                    for k in range(KC):
                        nc.tensor.matmul(psa, wt12[:, 0, k], h2[:, k],
                                         start=(k == 0), stop=(k == KC - 1))
                    psb = pmm.tile([P, R], F32, tag="ps")
                    for k in range(KC):
                        nc.tensor.matmul(psb, wt12[:, 1, k], h2[:, k],
                                         start=(k == 0), stop=(k == KC - 1))
                    if "bc" not in bc2_box:
                        pending.append((m, psa, psb))
                    else:
                        evict(m, psa, psb)
                    if m == 0:
                        finish_ss2()
                        rn2_box["rn"] = emit_rn(ss2_box["ss"])
                    if m == 1:
                        bc2_box["bc"] = emit_bcast(rn2_box["rn"])
                        for args in pending:
                            evict(*args)
                return g

            def emit_ffn_down(g, h2):
                cand = pB.tile([P, KC, R], BF16, tag="B")
                for m in range(KC):
                    wt3 = pW.tile([P, MH, P], BF16, tag="w3")
                    nc.sync.dma_start(out=wt3, in_=w3.ap()[m])
                    ps = pmm.tile([P, R], F32, tag="ps")
                    for k in range(MH):
                        nc.tensor.matmul(ps, wt3[:, k], g[:, k],
                                         start=(k == 0), stop=(k == MH - 1))
                    nc.vector.tensor_add(out=cand[:, m], in0=ps, in1=h2[:, m])
                return cand

            def emit_gate(b, cand, st, interleave):
                rows = slice(b * R, (b + 1) * R)
                for m in range(KC):
                    wtg = pW.tile([P, 2 * KC, P], BF16, tag="wg")
                    nc.sync.dma_start(out=wtg, in_=wg.ap()[m])
                    ps = pmm.tile([P, R], F32, tag="ps")
                    # state half first: st is ready long before cand, so the
                    # PE can start each gate chunk before FFN-down finishes.
                    for k in range(KC):
                        nc.tensor.matmul(ps, wtg[:, KC + k], st[:, k],
                                         start=(k == 0), stop=False)
                    for k in range(KC):
                        nc.tensor.matmul(ps, wtg[:, k], cand[:, k],
                                         start=False, stop=(k == KC - 1))
                    z = psm.tile([P, R], BF16, tag="z", bufs=1)
                    nc.scalar.activation(out=z, in_=ps, func=AF.Sigmoid,
                                         bias=(gb_sb[:, m:m + 1] if use_gb else 0.0))
                    d = psm.tile([P, R], BF16, tag="d", bufs=1)
                    nc.vector.tensor_tensor(d, cand[:, m], st[:, m], ALU.subtract)
                    nc.vector.tensor_tensor(d, z, d, ALU.mult)
                    y = psm.tile([P, R], F32, tag="y")
                    nc.vector.tensor_add(out=y, in0=d, in1=st[:, m])
                    nc.sync.dma_start(out=yT.ap()[m][:, rows], in_=y)
                    if m in interleave:
                        interleave[m]()

            # ---- software-pipelined block schedule: block b+1's input DMA,
            # h, and norm squares are traced inside block b's gate loop so
            # every engine's in-order stream interleaves the two blocks.
            nxt = {}

            def start_block(b):
                xt, st, parts = emit_input_dma(b)
                for p in parts:
                    p()
                res, hsq1 = emit_h_and_squares(b, xt, st)
                return {"st": st, "res": res, "hsq1": hsq1}

            cur = start_block(0)
            for b in range(NB):
                h2, ss2_box, finish_ss2 = emit_attn(b, cur["res"], cur["hsq1"])
                g = emit_ffn_up(h2, ss2_box, finish_ss2)
                cand = emit_ffn_down(g, h2)
                interleave = {}
                if b + 1 < NB:
                    xt_n, st_n, parts = emit_input_dma(b + 1)
                    state = {}

                    def hs(bn=b + 1, xt_n=xt_n, st_n=st_n, state=state):
                        state["res"], state["hsq1"] = emit_h_and_squares(bn, xt_n, st_n)

                    for i, pfn in enumerate(parts):
                        interleave[1 + i] = pfn
                    interleave[6] = hs
                    nxt = {"st": st_n, "state": state}
                emit_gate(b, cand, cur["st"], interleave)
                if b + 1 < NB:
                    cur = {"st": nxt["st"],
                           "res": nxt["state"]["res"],
                           "hsq1": nxt["state"]["hsq1"]}

    return _split_waits(nc)


_prog_cache: dict = {}


def _get_prog(use_bvo: bool, use_gb: bool) -> bass.Bass:
    key = (use_bvo, use_gb)
    if key not in _prog_cache:
        _prog_cache[key] = build_program(use_bvo, use_gb)
    return _prog_cache[key]


def prepare_inputs(x, state, g1, g2, in_proj_w, in_proj_b, out_proj_w, out_proj_b,
                   w1, w2, w3, gate_w, gate_b):
    """Host-side folding/packing. Returns (in_maps, use_bvo, use_gb)."""
    f32 = np.float32
    x = np.asarray(x, f32); state = np.asarray(state, f32)
    g1 = np.asarray(g1, f32); g2 = np.asarray(g2, f32)
    in_proj_w = np.asarray(in_proj_w, f32); in_proj_b = np.asarray(in_proj_b, f32)
    out_proj_w = np.asarray(out_proj_w, f32); out_proj_b = np.asarray(out_proj_b, f32)
    w1 = np.asarray(w1, f32); w2 = np.asarray(w2, f32); w3 = np.asarray(w3, f32)
    gate_w = np.asarray(gate_w, f32); gate_b = np.asarray(gate_b, f32)

    wv = in_proj_w[2 * DIM:]
    bv = in_proj_b[2 * DIM:]
    # attn == v (softmax over one key); fold v-proj + out-proj (and g1) together
    W_vo = (wv * g1[None, :]).T @ out_proj_w.T          # [K=2048, M=2048]
    b_vo = bv @ out_proj_w.T + out_proj_b               # [2048]
    W1 = np.zeros((DIM, HIDP), f32); W1[:, :HID] = (w1 * g2[None, :]).T
    W2 = np.zeros((DIM, HIDP), f32); W2[:, :HID] = (w2 * g2[None, :]).T
    W3 = np.zeros((HIDP, DIM), f32); W3[:HID] = w3.T
    WG = np.concatenate([gate_w[:, :DIM].T, gate_w[:, DIM:].T], axis=0)  # [4096, 2048]

    weights = {
        "wvo": _pack_lhsT(W_vo),
        "w12": np.ascontiguousarray(
            np.stack([_pack_lhsT(W1), _pack_lhsT(W2)], axis=2)),
        "w3": _pack_lhsT(W3),
        "wg": _pack_lhsT(WG),
    }
    use_bvo = bool(np.any(b_vo))
    use_gb = bool(np.any(gate_b))
    if use_bvo:
        weights["bvo"] = np.ascontiguousarray(b_vo.reshape(KC, P).T)
    if use_gb:
        weights["gb"] = np.ascontiguousarray(gate_b.reshape(KC, P).T)

    in_maps = []
    for c in range(NCORES):
        rs = slice(c * RPC, (c + 1) * RPC)
        m = dict(weights)
        m["xT"] = np.ascontiguousarray(x[rs].astype(nbf).T).reshape(KC, P, RPC)
        m["sT"] = np.ascontiguousarray(state[rs].astype(nbf).T).reshape(KC, P, RPC)
        in_maps.append(m)
    return in_maps, use_bvo, use_gb


def run(inputs: dict, trace: bool = False, trace_cores=None):
    in_maps, use_bvo, use_gb = prepare_inputs(**inputs)
    nc = _get_prog(use_bvo, use_gb)
    res = run_bass_kernel_spmd(
        nc, in_maps, core_ids=list(range(NCORES)),
        trace=trace, trace_cores=trace_cores)
    out = np.empty((BATCH, DIM), np.float32)
    for c in range(NCORES):
        yt = res.results[c]["yT"].reshape(DIM, RPC)
        out[c * RPC:(c + 1) * RPC] = yt.T
    return out, res


def kernel(**inputs) -> np.ndarray:
    out, _ = run(inputs, trace=False)
    return out
